# revision 3
# baseline (speedup 1.0000x reference)
"""Trainium2 Bass kernel for nn_CCL_Module (3x3 cost-volume softmax flow).

Reference computation (per batch):
  c1 = l2norm_C(feature1); wp = l2norm_C(feature2) zero-padded spatially.
  match_vol[d=(dh,dw)] = sum_C c1 * shift(wp, dh, dw)      (9 shifts, 3x3)
  p = softmax(10 * match_vol, over d)
  flow_w = sum_d p * dw ; flow_h = sum_d p * dh
  out = concat([flow_w, flow_h])  -> [B, 2, H, W]

Strategy (pure data parallel, one batch per NeuronCore, 8 cores):
  - SBUF layout: H=128 on partitions, free dims = (C=64, W).
  - dh shifts  -> three h-shifted copies of feature2 loaded by DMA.
  - dw shifts  -> free-dim AP offsets into w-padded tiles.
  - Raw (unnormalized) dots A_d = sum_C f1 * shift(f2) via DVE
    tensor_mul + strided tensor_reduce (reduce innermost = C).
  - L2 normalization folded into score scaling:
      score_d = 10 * A_d * rsqrt(|f1|^2) * rsqrt(|f2|^2 shifted)
  - Scores are bounded by |10| so softmax needs no max subtraction:
      flow = (sum_d w_d * exp(s_d)) / (sum_d exp(s_d))
"""

import numpy as np

B, C, H, W = 8, 64, 128, 128
N_CORES = 8
SOFTMAX_SCALE = 10.0

_CACHE = {}


def _build_program(repeat: int = 1, variant: str = "full"):
    import concourse.bass as bass
    import concourse.bacc as bacc
    import concourse.mybir as mybir
    from concourse.tile import TileContext
    from concourse.bass_utils import axon_active

    f32 = mybir.dt.float32
    nc = bacc.Bacc(
        "TRN2",
        target_bir_lowering=False,
        debug=not axon_active(),
        num_devices=N_CORES,
    )

    f1d = nc.declare_dram_parameter("feature1", [C, H, W], f32, isOutput=False)
    f2d = nc.declare_dram_parameter("feature2", [C, H, W], f32, isOutput=False)
    outd = nc.declare_dram_parameter("flow", [2, H, W], f32, isOutput=True)

    # DRAM views with h on the outer (partition) axis.
    f1v = f1d.rearrange("c h w -> h c w")
    f2v = f2d.rearrange("c h w -> h c w")
    outv = outd.rearrange("c h w -> h c w")

    # all-zero row used to zero-fill the dh edge partitions at load time
    zrow = nc.inline_tensor(np.zeros((1, C, W + 2), dtype=np.float32), name="zrow")

    with TileContext(nc) as tc:
        with tc.tile_pool(name="main", bufs=1) as pool:
          for _rep in range(repeat):
            # ---- input tiles ----
            xf1 = pool.tile([H, C, W], f32)          # f1, no padding
            # f2 with w padding (cols 0 and W+1), one tile per dh in {-1,0,1}.
            xf2_m = pool.tile([H, C, W + 2], f32)
            xf2_0 = pool.tile([H, C, W + 2], f32)
            xf2_p = pool.tile([H, C, W + 2], f32)

            nc.sync.dma_start(out=xf1[:, :, :], in_=f1v)
            # dh=0
            nc.sync.dma_start(out=xf2_0[:, :, 1 : W + 1], in_=f2v)
            # dh=-1: partition p holds f2 row p-1; row 0 is out of bounds -> 0
            nc.sync.dma_start(out=xf2_m[1:H, :, 1 : W + 1], in_=f2v[0 : H - 1])
            nc.sync.dma_start(out=xf2_m[0:1, :, :], in_=zrow[:])
            # dh=+1: partition p holds f2 row p+1; row H-1 out of bounds -> 0
            nc.sync.dma_start(out=xf2_p[0 : H - 1, :, 1 : W + 1], in_=f2v[1:H])
            nc.sync.dma_start(out=xf2_p[H - 1 : H, :, :], in_=zrow[:])

            # zero the w-pad columns so dw edge dots are exactly 0
            # (edge partitions already fully zeroed above; partition-0-based
            # memsets are legal for compute engines)
            for t in (xf2_m, xf2_0, xf2_p):
                nc.vector.memset(t[:, :, 0:1], 0.0)
                nc.vector.memset(t[:, :, W + 1 : W + 2], 0.0)

            xf2 = [xf2_m, xf2_0, xf2_p]

            # ---- raw correlation dots ----
            prod = pool.tile([H, C, W], f32)
            scoresA = pool.tile([H, 9, W], f32)     # A_d, d = dh*3+dw

            nmuls = 0 if variant == "loads" else 9
            for d in range(nmuls):
                dh, dw = d // 3 - 1, d % 3 - 1
                src = xf2[dh + 1][:, :, 1 + dw : 1 + dw + W]
                nc.vector.tensor_mul(prod[:, :, :], xf1[:, :, :], src)
                if variant == "muls":
                    continue
                # reduce over C (innermost after permute)
                nc.vector.tensor_reduce(
                    scoresA[:, d, :],
                    prod.rearrange("h c w -> h w c"),
                    axis=mybir.AxisListType.X,
                    op=mybir.AluOpType.add,
                )
            if variant in ("loads", "muls"):
                # consume every loaded tile so DCE can't drop the DMAs
                flows0 = pool.tile([H, 2, W], f32)
                nc.vector.tensor_add(flows0[:, 0, :], xf1[:, 0, :], xf2_m[:, 0, 0:W])
                nc.vector.tensor_add(flows0[:, 0, :], flows0[:, 0, :], xf2_0[:, 0, 0:W])
                nc.vector.tensor_add(flows0[:, 1, :], xf2_p[:, 0, 0:W], prod[:, 0, :])
                nc.sync.dma_start(out=outv, in_=flows0[:, :, :])
                continue

            # ---- norms ----
            r1sq = pool.tile([H, W], f32)
            r2m = pool.tile([H, W + 2], f32)  # |f2|^2 map, w-padded
            nc.vector.tensor_mul(prod[:, :, :], xf1[:, :, :], xf1[:, :, :])
            nc.vector.tensor_reduce(
                r1sq[:, :],
                prod.rearrange("h c w -> h w c"),
                axis=mybir.AxisListType.X,
                op=mybir.AluOpType.add,
            )
            f20 = xf2_0[:, :, 1 : W + 1]
            nc.vector.tensor_mul(prod[:, :, :], f20, f20)
            nc.vector.memset(r2m[:, 0:1], 1.0)
            nc.vector.memset(r2m[:, W + 1 : W + 2], 1.0)
            nc.vector.tensor_reduce(
                r2m[:, 1 : W + 1],
                prod.rearrange("h c w -> h w c"),
                axis=mybir.AxisListType.X,
                op=mybir.AluOpType.add,
            )

            # recip1 = 1/sqrt(r1sq), recip2 = 1/sqrt(r2m)
            recip1 = pool.tile([H, W], f32)
            recip2 = pool.tile([H, W + 2], f32)
            nc.scalar.sqrt(recip1[:, :], r1sq[:, :])
            nc.vector.reciprocal(recip1[:, :], recip1[:, :])
            nc.scalar.sqrt(recip2[:, :], r2m[:, :])
            nc.vector.reciprocal(recip2[:, :], recip2[:, :])

            # dh-shifted copies of recip2. Compute engines cannot address
            # partition-shifted APs, so shift across partitions via
            # SBUF->SBUF DMA. Edge rows clamp (their A is exactly 0).
            rec2_m = pool.tile([H, W + 2], f32)
            rec2_p = pool.tile([H, W + 2], f32)
            nc.sync.dma_start(out=rec2_m[1:H, :], in_=recip2[0 : H - 1, :])
            nc.sync.dma_start(out=rec2_m[0:1, :], in_=recip2[0:1, :])
            nc.sync.dma_start(out=rec2_p[0 : H - 1, :], in_=recip2[1:H, :])
            nc.sync.dma_start(out=rec2_p[H - 1 : H, :], in_=recip2[H - 1 : H, :])
            rec2 = [rec2_m, recip2, rec2_p]

            # ---- scores -> exp ----
            rmul = pool.tile([H, 9, W], f32)
            for d in range(9):
                dh, dw = d // 3 - 1, d % 3 - 1
                nc.vector.tensor_mul(
                    rmul[:, d, :], recip1[:, :], rec2[dh + 1][:, 1 + dw : 1 + dw + W]
                )
            expo = pool.tile([H, 9, W], f32)
            nc.vector.tensor_mul(rmul[:, :, :], rmul[:, :, :], scoresA[:, :, :])
            nc.scalar.activation(
                expo[:, :, :],
                rmul[:, :, :],
                mybir.ActivationFunctionType.Exp,
                scale=SOFTMAX_SCALE,
            )

            # ---- softmax-weighted displacement sums ----
            esum = pool.tile([H, W], f32)
            fwp = pool.tile([H, W], f32)
            fwm = pool.tile([H, W], f32)
            fhp = pool.tile([H, W], f32)
            fhm = pool.tile([H, W], f32)
            ex4 = expo.rearrange("h (a b) w -> h a b w", a=3)
            red = dict(axis=mybir.AxisListType.X, op=mybir.AluOpType.add)
            nc.vector.tensor_reduce(
                esum[:, :], expo.rearrange("h d w -> h w d"), **red
            )
            nc.vector.tensor_reduce(
                fwp[:, :], ex4[:, :, 2, :].rearrange("h a w -> h w a"), **red
            )
            nc.vector.tensor_reduce(
                fwm[:, :], ex4[:, :, 0, :].rearrange("h a w -> h w a"), **red
            )
            nc.vector.tensor_reduce(
                fhp[:, :], ex4[:, 2, :, :].rearrange("h b w -> h w b"), **red
            )
            nc.vector.tensor_reduce(
                fhm[:, :], ex4[:, 0, :, :].rearrange("h b w -> h w b"), **red
            )

            flows = pool.tile([H, 2, W], f32)
            nc.vector.reciprocal(esum[:, :], esum[:, :])
            nc.vector.tensor_sub(fwp[:, :], fwp[:, :], fwm[:, :])
            nc.vector.tensor_sub(fhp[:, :], fhp[:, :], fhm[:, :])
            nc.vector.tensor_mul(flows[:, 0, :], fwp[:, :], esum[:, :])
            nc.vector.tensor_mul(flows[:, 1, :], fhp[:, :], esum[:, :])

            nc.sync.dma_start(out=outv, in_=flows[:, :, :])

    nc.compile()
    return nc


def kernel(feature1: np.ndarray, feature2: np.ndarray) -> np.ndarray:
    from concourse import bass_utils

    if "nc" not in _CACHE:
        _CACHE["nc"] = _build_program()
    nc = _CACHE["nc"]

    f1 = np.ascontiguousarray(np.asarray(feature1, dtype=np.float32))
    f2 = np.ascontiguousarray(np.asarray(feature2, dtype=np.float32))
    in_maps = [
        {"feature1": f1[b], "feature2": f2[b]} for b in range(N_CORES)
    ]
    res = bass_utils.run_bass_kernel_spmd(nc, in_maps, list(range(N_CORES)))
    out = np.stack([res.results[b]["flow"] for b in range(N_CORES)], axis=0)
    return out.astype(np.float32)


def _ensure_ntff_hook():
    """Register the axon NTFF profile hook if antenv.axon_hooks is absent.

    The agent image lacks antenv.axon_hooks, so trn_boot never registered
    the hook; bass_utils imports it at trace time. Inject a shim module
    backed by the same ctypes hook trn_boot would have installed.
    """
    import sys, types

    try:
        from antenv.axon_hooks import get_axon_ntff_profile_hook  # noqa: F401

        return
    except ImportError:
        pass
    from trn_agent_boot.trn_boot import _ntff_profile_via_ctypes

    hook = _ntff_profile_via_ctypes("/opt/axon/libaxon_pjrt.so")
    mod = types.ModuleType("antenv.axon_hooks")
    mod.get_axon_ntff_profile_hook = lambda: hook
    mod.set_axon_ntff_profile_hook = lambda h: None
    sys.modules["antenv.axon_hooks"] = mod


def profile(feature1: np.ndarray, feature2: np.ndarray):
    """Profiled run: returns (exec_time_ns, trace_path)."""
    from concourse import bass_utils

    _ensure_ntff_hook()

    if "nc" not in _CACHE:
        _CACHE["nc"] = _build_program()
    nc = _CACHE["nc"]

    f1 = np.ascontiguousarray(np.asarray(feature1, dtype=np.float32))
    f2 = np.ascontiguousarray(np.asarray(feature2, dtype=np.float32))
    in_maps = [
        {"feature1": f1[b], "feature2": f2[b]} for b in range(N_CORES)
    ]
    res = bass_utils.run_bass_kernel_spmd(
        nc, in_maps, list(range(N_CORES)), trace=True
    )
    trace_path = None
    if res.instructions_and_trace is not None:
        trace_path = res.instructions_and_trace[1]
    return res.exec_time_ns, trace_path



# revision 10
# speedup vs baseline: 5.0345x; 5.0345x over previous
"""Trainium2 Bass kernel for nn_CCL_Module (3x3 cost-volume softmax flow).

Reference computation (per batch):
  c1 = l2norm_C(feature1); wp = l2norm_C(feature2) zero-padded spatially.
  match_vol[d=(dh,dw)] = sum_C c1 * shift(wp, dh, dw)      (9 shifts, 3x3)
  p = softmax(10 * match_vol, over d)
  flow_w = sum_d p * dw ; flow_h = sum_d p * dh
  out = concat([flow_w, flow_h])  -> [B, 2, H, W]

Strategy (pure data parallel, one batch per NeuronCore, 8 cores):
  - SBUF layout: partition p = s*64 + c (s = h-half, c = channel),
    free dim = flat (h, w) within the half. HBM loads are one contiguous
    ~32KB descriptor per partition (vs a [h, c, w] gather).
  - f2 loaded ONCE with a 1-row halo per half (66 rows); all 9 (dh, dw)
    shifts become free-dim offsets. Flat-shift w-wrap artifacts are
    zeroed in the score tiles (partition = w there).
  - Products f1*shift(f2) on DVE in bf16 (2x mode); an odd-aligned copy
    of f2 keeps dw=+-1 shifts 4B-aligned.
  - The C-reduction runs on the Tensor engine: per 128-pixel chunk,
    matmul(stationary = product chunk [128, 128], moving = half-masks
    [128, 2]) -> PSUM scores[w, 2h+s]. Norms reduce the same way from
    ScalarE squares.
  - Normalization folded into scores after the fact:
      score_d = 10 * A_d * rsqrt(|f1|^2) * rsqrt(|f2|^2 shifted)
    Scores are bounded by 10 so softmax needs no max subtraction:
      flow = (sum_d w_d * exp(s_d)) / (sum_d exp(s_d))
  - Softmax tail on DVE/ScalarE in [w, h] layout; final [h, w] layout
    restored with two TensorE transposes before the output DMA.
"""

import numpy as np

B, C, H, W = 8, 64, 128, 128
N_CORES = 8
SOFTMAX_SCALE = 10.0
HH = H // 2          # rows per half
FD = HH * W          # flat free size per half (8192)
FDP = (HH + 2) * W   # halo'd free size (8448)
FDP2 = FDP + 2       # plus 1 pad element each side in flat space

_CACHE = {}


def _build_program():
    import ml_dtypes
    import concourse.bass as bass
    import concourse.bacc as bacc
    import concourse.mybir as mybir
    from concourse.tile import TileContext
    from concourse.bass import MemorySpace
    from concourse.bass_utils import axon_active

    f32 = mybir.dt.float32
    bf16 = mybir.dt.bfloat16
    nc = bacc.Bacc(
        "TRN2",
        target_bir_lowering=False,
        debug=not axon_active(),
        num_devices=N_CORES,
    )

    f1d = nc.declare_dram_parameter("feature1", [C, H, W], f32, isOutput=False)
    f2d = nc.declare_dram_parameter("feature2", [C, H, W], f32, isOutput=False)
    outd = nc.declare_dram_parameter("flow", [2, H, W], f32, isOutput=True)

    v1 = f1d.rearrange("c h w -> c (h w)")   # [64, 16384]
    v2 = f2d.rearrange("c h w -> c (h w)")
    outv = outd.rearrange("j h w -> h j w")  # DMA dest: partition = h

    # half-masks for the partition-dim (channel) reduction on TensorE
    mask_np = np.zeros((128, 2), dtype=ml_dtypes.bfloat16)
    mask_np[:64, 0] = 1
    mask_np[64:, 1] = 1
    maskd = nc.inline_tensor(mask_np, name="halfmask")
    identd = nc.inline_tensor(np.eye(128, dtype=np.float32), name="ident")

    deint = dict(h=HH, s=2)  # free index 2h+s -> (s h) = global h

    with TileContext(nc) as tc:
        with tc.tile_pool(name="main", bufs=1) as pool, \
             tc.tile_pool(name="pbuf", bufs=2) as pbuf, \
             tc.tile_pool(name="psum", bufs=1, space=MemorySpace.PSUM) as psum:

            maskt = pool.tile([128, 2], bf16)
            ident = pool.tile([128, 128], f32)
            nc.sync.dma_start(out=maskt[:, :], in_=maskd[:, :])
            nc.sync.dma_start(out=ident[:, :], in_=identd[:, :])

            # ---- staged fp32 loads (contiguous per partition) ----
            with tc.tile_pool(name="stage", bufs=1) as stage:
                F1f = stage.tile([128, FD], f32)
                F2f = stage.tile([128, FDP2], f32)
                nc.sync.dma_start(out=F1f[0:64, :], in_=v1[:, 0:FD])
                nc.sync.dma_start(out=F1f[64:128, :], in_=v1[:, FD : 2 * FD])
                # halo'd f2 at flat offset 1: partition s*64+c holds rows
                # 64s-1 .. 64s+64; pad rows and flat-edge elements zeroed
                nc.vector.memset(F2f[0:64, 0 : 1 + W], 0.0)        # h = -1
                nc.vector.memset(F2f[64:128, 0:1], 0.0)
                nc.vector.memset(F2f[64:128, 1 + FDP - W : FDP2], 0.0)  # h = 128
                nc.vector.memset(F2f[0:64, 1 + FDP : FDP2], 0.0)
                nc.sync.dma_start(
                    out=F2f[0:64, 1 + W : 1 + FDP], in_=v2[:, 0 : FD + W]
                )
                nc.sync.dma_start(
                    out=F2f[64:128, 1 : 1 + FDP - W], in_=v2[:, FD - W : 2 * FD]
                )

                F1 = pool.tile([128, FD], bf16)
                F2 = pool.tile([128, FDP2], bf16)
                nc.vector.tensor_copy(F1[:, :], F1f[:, :])
                nc.vector.tensor_copy(F2[:, :], F2f[:, :])

            # odd-aligned copy for dw = +-1 products: F2o[i] = F2[i+1]
            F2o = pool.tile([128, FDP2], bf16)
            nc.sync.dma_start(out=F2o[:, 0 : FDP2 - 1], in_=F2[:, 1:FDP2])
            nc.vector.memset(F2o[:, FDP2 - 1 : FDP2], 0.0)

            # squares for the norms (ScalarE, parallel to DVE)
            S1 = pool.tile([128, FD], bf16)
            S2 = pool.tile([128, FD], bf16)
            nc.scalar.square(S1[:, :], F1[:, :])
            nc.scalar.square(S2[:, :], F2[:, 1 + W : 1 + W + FD])

            # PSUM score tiles: 3 banks x 4 slots of [128, 128] fp32
            T0 = psum.tile([128, 4, 128], f32)
            T1 = psum.tile([128, 4, 128], f32)
            T2 = psum.tile([128, 4, 128], f32)

            def slot(i):
                t = (T0, T1, T2)[i // 4]
                return t[:, i % 4, :]

            # ---- products (DVE) + channel-reduction matmuls (PE) ----
            for d in range(9):
                dh, dw = d // 3 - 1, d % 3 - 1
                base = 1 + (1 + dh) * W + dw
                if base % 2 == 0:
                    src2 = F2[:, base : base + FD]
                else:
                    src2 = F2o[:, base - 1 : base - 1 + FD]
                P = pbuf.tile([128, FD], bf16, tag="P")
                nc.vector.tensor_mul(P[:, :], F1[:, :], src2)
                # flat w-shift wrap fixup: zero the wrapped product column
                # so edge dots are exactly 0 (partition-offset memsets on
                # PSUM are illegal, so fix the products, not the scores)
                Pv = P.rearrange("p (k w) -> p k w", w=W)
                if dw == 1:
                    nc.vector.memset(Pv[:, :, W - 1 : W], 0.0)
                elif dw == -1:
                    nc.vector.memset(Pv[:, :, 0:1], 0.0)
                out_d = slot(d)
                for k in range(64):
                    nc.tensor.matmul(
                        out_d[:, 2 * k : 2 * k + 2],
                        P[:, 128 * k : 128 * (k + 1)],
                        maskt[:, :],
                    )

            # ---- norm reductions (PE) ----
            n1 = slot(9)
            n2 = slot(10)
            for k in range(64):
                nc.tensor.matmul(
                    n1[:, 2 * k : 2 * k + 2],
                    S1[:, 128 * k : 128 * (k + 1)],
                    maskt[:, :],
                )
                nc.tensor.matmul(
                    n2[:, 2 * k : 2 * k + 2],
                    S2[:, 128 * k : 128 * (k + 1)],
                    maskt[:, :],
                )

            # rec = 1/sqrt(n), de-interleaved to [w, h_global]
            rec1 = pool.tile([128, 128], f32)
            rec2 = pool.tile([128, 128], f32)
            nc.scalar.sqrt(
                rec1.rearrange("w (s h) -> w s h", s=2),
                n1.rearrange("w (h s) -> w s h", s=2),
            )
            nc.scalar.sqrt(
                rec2.rearrange("w (s h) -> w s h", s=2),
                n2.rearrange("w (h s) -> w s h", s=2),
            )
            nc.vector.reciprocal(rec1[:, :], rec1[:, :])
            nc.vector.reciprocal(rec2[:, :], rec2[:, :])

            # rec2 shifted by dw across partitions (SBUF->SBUF DMA, edge clamp)
            R2wm = pool.tile([128, 128], f32)  # value at w-1
            R2wp = pool.tile([128, 128], f32)  # value at w+1
            nc.sync.dma_start(out=R2wm[1:128, :], in_=rec2[0:127, :])
            nc.sync.dma_start(out=R2wm[0:1, :], in_=rec2[0:1, :])
            nc.sync.dma_start(out=R2wp[0:127, :], in_=rec2[1:128, :])
            nc.sync.dma_start(out=R2wp[127:128, :], in_=rec2[127:128, :])
            byw = {-1: R2wm, 0: rec2, 1: R2wp}

            # rec2 shifted by dh along free dim (h_global contiguous, clamped)
            r2v = {}
            for dw in (-1, 0, 1):
                src = byw[dw]
                r2v[(0, dw)] = src
                vp = pool.tile([128, 128], f32, tag=f"r2p{dw}")
                nc.vector.tensor_copy(vp[:, 0:127], src[:, 1:128])
                nc.vector.tensor_copy(vp[:, 127:128], src[:, 127:128])
                r2v[(1, dw)] = vp
                vm = pool.tile([128, 128], f32, tag=f"r2m{dw}")
                nc.vector.tensor_copy(vm[:, 1:128], src[:, 0:127])
                nc.vector.tensor_copy(vm[:, 0:1], src[:, 0:1])
                r2v[(-1, dw)] = vm

            # ---- scores -> exp ----
            t1 = pool.tile([128, 9, 128], f32)
            for d in range(9):
                dh, dw = d // 3 - 1, d % 3 - 1
                nc.vector.tensor_mul(
                    t1[:, d, :].rearrange("w (s h) -> w s h", s=2),
                    slot(d).rearrange("w (h s) -> w s h", s=2),
                    r2v[(dh, dw)].rearrange("w (s h) -> w s h", s=2),
                )
            for d in range(9):
                nc.vector.tensor_mul(t1[:, d, :], t1[:, d, :], rec1[:, :])
            expo = pool.tile([128, 9, 128], f32)
            nc.scalar.activation(
                expo[:, :, :],
                t1[:, :, :],
                mybir.ActivationFunctionType.Exp,
                scale=SOFTMAX_SCALE,
            )

            # ---- softmax-weighted displacement sums ----
            esum = pool.tile([128, 128], f32)
            fwp = pool.tile([128, 128], f32)
            fwm = pool.tile([128, 128], f32)
            fhp = pool.tile([128, 128], f32)
            fhm = pool.tile([128, 128], f32)
            ex4 = expo.rearrange("w (a b) h -> w a b h", a=3)
            red = dict(axis=mybir.AxisListType.X, op=mybir.AluOpType.add)
            nc.vector.tensor_reduce(
                esum[:, :], expo.rearrange("w d h -> w h d"), **red
            )
            nc.vector.tensor_reduce(
                fwp[:, :], ex4[:, :, 2, :].rearrange("w a h -> w h a"), **red
            )
            nc.vector.tensor_reduce(
                fwm[:, :], ex4[:, :, 0, :].rearrange("w a h -> w h a"), **red
            )
            nc.vector.tensor_reduce(
                fhp[:, :], ex4[:, 2, :, :].rearrange("w b h -> w h b"), **red
            )
            nc.vector.tensor_reduce(
                fhm[:, :], ex4[:, 0, :, :].rearrange("w b h -> w h b"), **red
            )

            flows = pool.tile([128, 2, 128], f32)
            nc.vector.reciprocal(esum[:, :], esum[:, :])
            nc.vector.tensor_sub(fwp[:, :], fwp[:, :], fwm[:, :])
            nc.vector.tensor_sub(fhp[:, :], fhp[:, :], fhm[:, :])
            nc.vector.tensor_mul(flows[:, 0, :], fwp[:, :], esum[:, :])
            nc.vector.tensor_mul(flows[:, 1, :], fhp[:, :], esum[:, :])

            # ---- transpose [w, h] -> [h, w] and write out ----
            TF = psum.tile([128, 2, 128], f32)
            nc.tensor.transpose(TF[:, 0, :], flows[:, 0, :], ident[:, :])
            nc.tensor.transpose(TF[:, 1, :], flows[:, 1, :], ident[:, :])
            flowT = pool.tile([128, 2, 128], f32)
            nc.vector.tensor_copy(flowT[:, :, :], TF[:, :, :])
            nc.sync.dma_start(out=outv, in_=flowT[:, :, :])

    nc.compile()
    return nc


def kernel(feature1: np.ndarray, feature2: np.ndarray) -> np.ndarray:
    from concourse import bass_utils

    if "nc" not in _CACHE:
        _CACHE["nc"] = _build_program()
    nc = _CACHE["nc"]

    f1 = np.ascontiguousarray(np.asarray(feature1, dtype=np.float32))
    f2 = np.ascontiguousarray(np.asarray(feature2, dtype=np.float32))
    in_maps = [
        {"feature1": f1[b], "feature2": f2[b]} for b in range(N_CORES)
    ]
    res = bass_utils.run_bass_kernel_spmd(nc, in_maps, list(range(N_CORES)))
    out = np.stack([res.results[b]["flow"] for b in range(N_CORES)], axis=0)
    return out.astype(np.float32)


def _ensure_ntff_hook():
    """Register the axon NTFF profile hook if antenv.axon_hooks is absent.

    The agent image lacks antenv.axon_hooks, so trn_boot never registered
    the hook; bass_utils imports it at trace time. Inject a shim module
    backed by the same ctypes hook trn_boot would have installed.
    """
    import sys, types

    try:
        from antenv.axon_hooks import get_axon_ntff_profile_hook  # noqa: F401

        return
    except ImportError:
        pass
    from trn_agent_boot.trn_boot import _ntff_profile_via_ctypes

    hook = _ntff_profile_via_ctypes("/opt/axon/libaxon_pjrt.so")
    mod = types.ModuleType("antenv.axon_hooks")
    mod.get_axon_ntff_profile_hook = lambda: hook
    mod.set_axon_ntff_profile_hook = lambda h: None
    sys.modules["antenv.axon_hooks"] = mod


def profile(feature1: np.ndarray, feature2: np.ndarray):
    """Profiled run: returns (exec_time_ns, trace_path)."""
    from concourse import bass_utils

    _ensure_ntff_hook()

    if "nc" not in _CACHE:
        _CACHE["nc"] = _build_program()
    nc = _CACHE["nc"]

    f1 = np.ascontiguousarray(np.asarray(feature1, dtype=np.float32))
    f2 = np.ascontiguousarray(np.asarray(feature2, dtype=np.float32))
    in_maps = [
        {"feature1": f1[b], "feature2": f2[b]} for b in range(N_CORES)
    ]
    res = bass_utils.run_bass_kernel_spmd(
        nc, in_maps, list(range(N_CORES)), trace=True
    )
    trace_path = None
    if res.instructions_and_trace is not None:
        trace_path = res.instructions_and_trace[1]
    return res.exec_time_ns, trace_path
